# revision 16
# baseline (speedup 1.0000x reference)
"""Causal self-attention with RoPE on 8 Trainium2 NeuronCores.

Sharding: core c = 4*b + g handles batch b (of 2) and head group g (4 of 16
heads). Each core computes q/k/v projections for its heads, head-local causal
softmax attention, and a partial output projection (Wp columns of its heads);
the host sums the 4 partials per batch and adds bp.

v2 pipeline: all phases share one PSUM bank plan (qv 2 + pst 4 + pav 2 = 8
banks) and are emitted interleaved so the list scheduler overlaps them:

  loads; qk(q0); v(0,1); attn(c=0); qk(q1); v(2,3); attn(c=1); norm+proj(0);
  qk(q2); v(4,5); attn(c=2); np(1); qk(q3); v(6,7); attn(c=3); np(2); np(3)

Attention runs in 512-wide t passes (c = pass = output chunk).  For each
(head-pair jt, pass c, s-tile k) the two heads' S^T tiles go into one
[128,1024] PSUM tile via row-tiled concurrent K=64 matmuls (tile_position
(0,0)/(64,0)), so one EXP instruction covers both heads.  P^T (bf16, SBUF)
feeds per-head AV matmuls whose 65th row accumulates the softmax denominator
(ones column in v''); the denominator row is copied straight into rr4 with a
partition-offset DVE copy (no DMA staging).  Norm inverts rr4 once per chunk,
broadcasts via a K=4 indicator matmul, and scales yT in place; proj then
writes bf16 partials that are DMA'd per-chunk as single descriptors.
"""

import sys

for _p in ("/opt/trn_rl_repo",):
    if _p not in sys.path:
        sys.path.append(_p)

import numpy as np
import ml_dtypes
from contextlib import ExitStack

import concourse.bacc as bacc
import concourse.tile as tile
from concourse import mybir
from concourse.bass_utils import run_bass_kernel_spmd

F32 = mybir.dt.float32
F32R = mybir.dt.float32r
BF16 = mybir.dt.bfloat16
EXP = mybir.ActivationFunctionType.Exp

B, T, C = 2, 2048, 1024
H, D = 16, 64
HG = 4                 # heads per core
JG = HG * D            # 256 j-columns per core
VW = HG * 65           # v'' width (64 dims + ones col per head)
NKT = C // 128         # 8 contraction tiles
NTT = T // 128         # 16 t-tiles / s-tiles
NC4 = T // 512         # 4 512-chunks
SCALE = 1.0 / np.sqrt(D)

_NC_CACHE = None


def build_bass(zero_bias=False):
    nc = bacc.Bacc()

    xT = nc.declare_dram_parameter("xT", [C, T], BF16, isOutput=False)
    wqT = nc.declare_dram_parameter("wqT", [C, JG], BF16, isOutput=False)
    wkT = nc.declare_dram_parameter("wkT", [C, JG], BF16, isOutput=False)
    wvT = nc.declare_dram_parameter("wvT", [C, JG], BF16, isOutput=False)
    wpT = nc.declare_dram_parameter("wpT", [JG, C], BF16, isOutput=False)
    cosT = nc.declare_dram_parameter("cosT", [128, T], BF16, isOutput=False)
    ssT = nc.declare_dram_parameter("ssT", [128, T], BF16, isOutput=False)
    bmask = nc.declare_dram_parameter("bmask", [128, 384], BF16, isOutput=False)
    ind4 = nc.declare_dram_parameter("ind4", [4, JG], F32, isOutput=False)
    if not zero_bias:
        bq_r = nc.declare_dram_parameter("bq_r", [1, JG], F32, isOutput=False)
        bk_r = nc.declare_dram_parameter("bk_r", [1, JG], F32, isOutput=False)
        bv_r = nc.declare_dram_parameter("bv_r", [1, JG], F32, isOutput=False)
        ones_r = nc.declare_dram_parameter("ones_r", [1, 512], F32, isOutput=False)

    outT = nc.declare_dram_parameter("outT", [NC4 * C, 512], BF16, isOutput=True)

    with (
        tile.TileContext(nc) as tc,
        ExitStack() as ctx,
        nc.allow_low_precision(reason="f32r/bf16 matmul pipeline"),
    ):
        consts = ctx.enter_context(tc.tile_pool(name="consts", bufs=1))

        def wtile(name):
            return consts.tile([128, NKT * JG], BF16, tag=name, name=name)

        wq_sb, wk_sb, wv_sb = wtile("wq"), wtile("wk"), wtile("wv")

        def load_w(eng, t, dram):
            eng.dma_start(
                t[:].rearrange("p (i j) -> p i j", i=NKT),
                dram[:].rearrange("(i p) j -> p i j", p=128),
            )

        # x resident: 8 tiles [128, T], loaded in halves (sync: half 0,
        # vector: half 1) so quarter-0 matmuls can start early.
        xsb = [
            consts.tile([128, T], BF16, tag=f"x{i}", name=f"x{i}")
            for i in range(NKT)
        ]

        def load_x(eng, i, q):
            eng.dma_start(
                xsb[i][:, 512 * q : 512 * (q + 1)],
                xT[128 * i : 128 * (i + 1), 512 * q : 512 * (q + 1)],
            )

        def load_w_ctile(eng, t, dram, i):
            eng.dma_start(
                t[:, i * JG : (i + 1) * JG],
                dram[128 * i : 128 * (i + 1), :],
            )

        # Fine-grained descriptors ordered by first consumption (the HW DMA
        # rings fair-share all SW queues, so one big descriptor completes
        # only when the whole early load set drains).
        # sync: x evens (q0,q1 interleaved per i), then evens q2,q3
        for i in (0, 2, 4, 6):
            load_x(nc.sync, i, 0)
            load_x(nc.sync, i, 1)
        # scalar: weight ctiles evens, then per-odd-i x+weights, then x odds q2/q3
        for i in (0, 2, 4, 6):
            load_w_ctile(nc.scalar, wq_sb, wqT, i)
            load_w_ctile(nc.scalar, wk_sb, wkT, i)
        for i in (1, 3, 5, 7):
            load_x(nc.scalar, i, 0)
            load_x(nc.scalar, i, 1)
            load_w_ctile(nc.scalar, wq_sb, wqT, i)
            load_w_ctile(nc.scalar, wk_sb, wkT, i)
        for q in (2, 3):
            for i in (0, 2, 4, 6):
                load_x(nc.sync, i, q)
            for i in (1, 3, 5, 7):
                load_x(nc.scalar, i, q)
        # gpsimd: rope tables first (needed ~12us), wv ctiles (v phase),
        # small consts, wp last (needed only by the first proj ~60us).
        cos_sb = consts.tile([128, T], BF16, tag="cos", name="cos")
        ss_sb = consts.tile([128, T], BF16, tag="ss", name="ss")
        nc.gpsimd.dma_start(cos_sb[:], cosT[:])
        nc.gpsimd.dma_start(ss_sb[:], ssT[:])
        for i in range(NKT):
            load_w_ctile(nc.gpsimd, wv_sb, wvT, i)
        bm_sb = consts.tile([128, 384], BF16, tag="bmask", name="bmask")
        nc.gpsimd.dma_start(bm_sb[:], bmask[:])
        ind4_sb = consts.tile([4, JG], F32R, tag="ind4", name="ind4")
        nc.gpsimd.dma_start(ind4_sb[:], ind4[:].bitcast(F32R))
        wp_sb = [None, None]
        for jt in range(2):
            wp_sb[jt] = consts.tile([128, C], BF16, tag=f"wp{jt}", name=f"wp{jt}")
            nc.gpsimd.dma_start(
                wp_sb[jt][:], wpT[128 * jt : 128 * (jt + 1), :]
            )
        if not zero_bias:
            def load_const(name, dram, shape):
                t = consts.tile(shape, F32R, tag=name, name=name)
                nc.gpsimd.dma_start(t[:], dram[:].bitcast(F32R))
                return t

            bq_sb = load_const("bq", bq_r, [1, JG])
            bk_sb = load_const("bk", bk_r, [1, JG])
            bv_sb = load_const("bv", bv_r, [1, JG])
            ones_sb = load_const("ones", ones_r, [1, 512])

        qkv_sb = ctx.enter_context(tc.tile_pool(name="qkv", bufs=1))
        qT_sb = [qkv_sb.tile([128, T], BF16, tag=f"qT{j}", name=f"qT{j}") for j in range(2)]
        kT_sb = [qkv_sb.tile([128, T], BF16, tag=f"kT{j}", name=f"kT{j}") for j in range(2)]
        yT_sb = [qkv_sb.tile([128, T], BF16, tag=f"yT{j}", name=f"yT{j}") for j in range(2)]
        v_sb = [qkv_sb.tile([128, VW], BF16, tag=f"v{s}", name=f"v{s}") for s in range(NTT)]
        rr4_sb = qkv_sb.tile([4, T], F32R, tag="rr4", name="rr4")
        rr4i_sb = qkv_sb.tile([4, T], F32R, tag="rr4i", name="rr4i")
        rscr_sb = qkv_sb.tile([1, 2048], F32, tag="rscr", name="rscr")
        rscr2_sb = qkv_sb.tile([4, T], F32, tag="rscr2", name="rscr2")

        # ones column of v'' (row 64 of each head's AV result = denominator)
        for s in range(NTT):
            nc.gpsimd.memset(
                v_sb[s][:].rearrange("p (h w) -> p h w", h=HG)[:, :, 64:65], 1.0
            )

        rope_pool = ctx.enter_context(tc.tile_pool(name="rope", bufs=4))
        ppt_pool = ctx.enter_context(tc.tile_pool(name="pptp", bufs=4))
        ostage = ctx.enter_context(tc.tile_pool(name="ostage", bufs=2))
        ps = ctx.enter_context(tc.tile_pool(name="ps", bufs=2, space="PSUM"))

        # ---- qk phase: per T-half, 4 slots (q-j0, k-j0, q-j1, k-j1); each
        # weight slice is loaded once and used for both quarters of the half.
        def qk_half(hf, slots):
            iorder = (0, 2, 4, 6, 1, 3, 5, 7) if hf == 0 else tuple(range(NKT))
            slot_info = (
                (0, wq_sb, "bq", qT_sb), (0, wk_sb, "bk", kT_sb),
                (1, wq_sb, "bq", qT_sb), (1, wk_sb, "bk", kT_sb),
            )
            for sl in slots:
                jt, w_sb, bias_key, dst = slot_info[sl]
                p2 = [
                    ps.tile([128, 512], F32, tag="qv", name="pqk")
                    for _ in range(2)
                ]
                for step, i in enumerate(iorder):
                    for qh in range(2):
                        nc.tensor.matmul(
                            p2[qh][:],
                            w_sb[:, i * JG + 128 * jt : i * JG + 128 * (jt + 1)],
                            xsb[i][:, 1024 * hf + 512 * qh : 1024 * hf + 512 * (qh + 1)],
                            start=(step == 0),
                            stop=(zero_bias and step == NKT - 1),
                        )
                if not zero_bias:
                    b_sb = bq_sb if bias_key == "bq" else bk_sb
                    for qh in range(2):
                        nc.tensor.matmul(
                            p2[qh][:],
                            b_sb[:, 128 * jt : 128 * (jt + 1)],
                            ones_sb[:, :],
                            start=False,
                            stop=True,
                        )
                # RoPE: dst = p*cos + rotate_half(p)*ss
                for qh in range(2):
                    tlo = 1024 * hf + 512 * qh
                    p = p2[qh]
                    out = dst[jt][:, tlo : tlo + 512]
                    qtmp = rope_pool.tile([128, 512], BF16, tag="qtmp", name="qtmp")
                    nc.vector.tensor_copy(qtmp[:], p[:])
                    rin = rope_pool.tile([128, 512], BF16, tag="rin", name="rin")
                    for h0 in (0, 64):
                        a_, b_, c_ = h0, h0 + 32, h0 + 64
                        dq = (nc.gpsimd, nc.sync)[(sl + qh) % 2]
                        dq.dma_start(rin[a_:b_, :], qtmp[b_:c_, :])
                        dq.dma_start(rin[b_:c_, :], qtmp[a_:b_, :])
                    nc.vector.tensor_mul(out, qtmp[:], cos_sb[:, tlo : tlo + 512])
                    rot = rope_pool.tile([128, 512], BF16, tag="rot", name="rot")
                    nc.vector.tensor_mul(rot[:], rin[:], ss_sb[:, tlo : tlo + 512])
                    nc.vector.tensor_add(out, out, rot[:])

        # ---- v phase: group g covers t-tiles 2g, 2g+1 -------------------
        def v_group(g):
            qtr = g // 2
            pvt = ps.tile([128, 512], F32, tag="qv", name="pv")
            for t2 in range(2):
                tt4 = 2 * (g % 2) + t2
                sl = pvt[:, JG * t2 : JG * (t2 + 1)]
                for i in range(NKT):
                    nc.tensor.matmul(
                        sl,
                        xsb[i][:, 512 * qtr + 128 * tt4 : 512 * qtr + 128 * (tt4 + 1)],
                        wv_sb[:, i * JG : (i + 1) * JG],
                        start=(i == 0),
                        stop=(zero_bias and i == NKT - 1),
                    )
                if not zero_bias:
                    nc.tensor.matmul(
                        sl, ones_sb[:, :128], bv_sb[:, :], start=False, stop=True
                    )
            for t2 in range(2):
                tt = 4 * qtr + 2 * (g % 2) + t2
                vv = v_sb[tt][:].rearrange("p (h w) -> p h w", h=HG)
                nc.vector.tensor_copy(
                    vv[:, :, 0:64],
                    pvt[:, JG * t2 : JG * (t2 + 1)].rearrange(
                        "p (h w) -> p h w", h=HG
                    ),
                )

        # ---- attention: head pair jt, 512-wide t pass c -----------------
        def attn(jt, c):
            K = 4 * c + 4          # s-tiles 0..4c+3
            tl = 512 * c
            pavs = [
                ps.tile([128, 512], F32, tag="pav", name=f"pav{m}")
                for m in range(2)
            ]
            ppts = [None] * K

            def do_s(k):
                a = max(128 * k - tl, 0)
                pst = ps.tile([128, 1024], F32, tag="pst", name="pst")
                for m in range(2):
                    nc.tensor.matmul(
                        pst[:, 512 * m + a : 512 * (m + 1)],
                        kT_sb[jt][64 * m : 64 * (m + 1), 128 * k : 128 * (k + 1)],
                        qT_sb[jt][64 * m : 64 * (m + 1), tl + a : tl + 512],
                        start=True,
                        stop=True,
                        tile_position=(64 * m, 0),
                    )
                pt = ppt_pool.tile([128, 1024], BF16, tag="ppt", name="ppt")
                if a:
                    src = pst.rearrange("p (h w) -> p h w", h=2)[:, :, a:512]
                    dst = pt.rearrange("p (h w) -> p h w", h=2)[:, :, a:512]
                else:
                    src, dst = pst[:], pt[:]
                nc.scalar.activation(dst, src, EXP, scale=float(SCALE))
                if k >= 4 * c:  # diagonal block: causal mask post-exp
                    for m in range(2):
                        blk = pt[:, 512 * m + a : 512 * m + a + 128]
                        nc.vector.tensor_mul(blk, blk, bm_sb[:, 256:384])
                ppts[k] = pt

            def do_av(k):
                a = max(128 * k - tl, 0)
                for m in range(2):
                    nc.tensor.matmul(
                        pavs[m][0:65, a:512],
                        v_sb[k][:, 65 * (2 * jt + m) : 65 * (2 * jt + m) + 65],
                        ppts[k][:, 512 * m + a : 512 * (m + 1)],
                        start=(k == 0),
                        stop=(k == K - 1),
                    )

            for k in range(K):
                do_s(k)
                if k:
                    do_av(k - 1)
            do_av(K - 1)
            for m in range(2):
                nc.vector.tensor_copy(
                    yT_sb[jt][64 * m : 64 * (m + 1), tl : tl + 512],
                    pavs[m][0:64, :],
                )
                r = 2 * jt + m
                nc.vector.tensor_copy(
                    rscr_sb[0:1, 512 * r : 512 * (r + 1)],
                    pavs[m][64:65, :],
                )
                (nc.sync, nc.gpsimd)[r % 2].dma_start(
                    rr4_sb[r : r + 1, tl : tl + 512],
                    rscr_sb[0:1, 512 * r : 512 * (r + 1)].bitcast(F32R),
                )

        # ---- norm (yT *= 1/r per head) + output projection --------------
        def np_chunk(c4):
            cl = 512 * c4
            nc.vector.reciprocal_approx_fast(
                out=rscr2_sb[:, cl : cl + 512],
                in_=rr4_sb[:, cl : cl + 512].bitcast(F32),
            )
            nc.sync.dma_start(
                rr4i_sb[:, cl : cl + 512],
                rscr2_sb[:, cl : cl + 512].bitcast(F32R),
            )
            for jt in range(2):
                pn = ps.tile([128, 512], F32, tag="pav", name="pn")
                nc.tensor.matmul(
                    pn[:],
                    ind4_sb[:, 128 * jt : 128 * (jt + 1)],
                    rr4i_sb[:, cl : cl + 512],
                    start=True,
                    stop=True,
                )
                sl = yT_sb[jt][:, cl : cl + 512]
                nc.vector.tensor_mul(sl, sl, pn[:])
            ost = ostage.tile([128, 8 * 512], BF16, tag="ost", name="ost")
            for et in range(8):
                pp = ps.tile([128, 512], F32, tag="pav", name="pp")
                for jt in range(2):
                    nc.tensor.matmul(
                        pp[:],
                        wp_sb[jt][:, 128 * et : 128 * (et + 1)],
                        yT_sb[jt][:, cl : cl + 512],
                        start=(jt == 0),
                        stop=(jt == 1),
                    )
                if c4 == 3 and et % 2 == 0:
                    nc.scalar.copy(ost[:, 512 * et : 512 * (et + 1)], pp[:])
                else:
                    nc.vector.tensor_copy(ost[:, 512 * et : 512 * (et + 1)], pp[:])
            (nc.sync, nc.gpsimd)[c4 % 2].dma_start(
                outT[c4 * C : (c4 + 1) * C, :].rearrange("(e p) t -> p e t", p=128),
                ost[:].rearrange("p (e t) -> p e t", e=8),
            )

        # ---- emission schedule ------------------------------------------
        qk_half(0, (0, 1))
        v_group(0)
        v_group(1)
        attn(0, 0)
        qk_half(0, (2, 3))
        attn(1, 0)
        qk_half(1, (0, 1))
        v_group(2)
        v_group(3)
        attn(0, 1)
        qk_half(1, (2, 3))
        attn(1, 1)
        np_chunk(0)
        v_group(4)
        v_group(5)
        attn(0, 2)
        attn(1, 2)
        np_chunk(1)
        v_group(6)
        v_group(7)
        attn(0, 3)
        np_chunk(2)
        attn(1, 3)
        np_chunk(3)

    nc.finalize()
    return nc


def _rope_tables():
    inv_freq = 1.0 / (10000.0 ** (np.arange(0, D, 2, dtype=np.float32) / D))
    t = np.arange(T, dtype=np.float32)
    freqs = t[:, None] * inv_freq[None, :]              # [T, 32]
    emb = np.concatenate([freqs, freqs], axis=1)        # [T, 64]
    cos = np.cos(emb).astype(np.float32).T              # [64, T]
    sin = np.sin(emb).astype(np.float32).T              # [64, T]
    # rotate_half signs at destination rows: rot[d<32] = -q[d+32]*sin[d]
    ss = np.concatenate([-sin[:32], sin[32:]], axis=0)
    cosT = np.concatenate([cos, cos], axis=0)           # [128, T] (2 heads)
    ssT = np.concatenate([ss, ss], axis=0)              # [128, T]
    return (
        np.ascontiguousarray(cosT).astype(ml_dtypes.bfloat16),
        np.ascontiguousarray(ssT).astype(ml_dtypes.bfloat16),
    )


def _host_inputs(x, Wq, bq, Wk, bk, Wv, bv, Wp, bp, zero_bias):
    cosT, ssT = _rope_tables()
    s = np.arange(128)[:, None]
    u = np.arange(384)[None, :]
    bmask = ((u - 256) >= s).astype(ml_dtypes.bfloat16)
    ind4 = np.zeros((4, JG), np.float32)
    for j in range(JG):
        ind4[2 * (j // 128) + (j % 128) // 64, j] = 1.0

    maps = []
    for b in range(B):
        for g in range(4):
            J = slice(g * JG, (g + 1) * JG)
            m = {
                "xT": np.ascontiguousarray(x[b].T).astype(ml_dtypes.bfloat16),
                "wqT": np.ascontiguousarray(Wq[J, :].T).astype(ml_dtypes.bfloat16),
                "wkT": np.ascontiguousarray(Wk[J, :].T).astype(ml_dtypes.bfloat16),
                "wvT": np.ascontiguousarray(Wv[J, :].T).astype(ml_dtypes.bfloat16),
                "wpT": np.ascontiguousarray(Wp[:, J].T).astype(ml_dtypes.bfloat16),
                "cosT": cosT,
                "ssT": ssT,
                "bmask": bmask,
                "ind4": ind4,
            }
            if not zero_bias:
                m["bq_r"] = bq[None, J].astype(np.float32)
                m["bk_r"] = bk[None, J].astype(np.float32)
                m["bv_r"] = bv[None, J].astype(np.float32)
                m["ones_r"] = np.ones((1, 512), np.float32)
            maps.append(m)
    return maps


def kernel(x, Wq, bq, Wk, bk, Wv, bv, Wp, bp, _trace=False):
    global _NC_CACHE
    x, Wq, bq, Wk, bk, Wv, bv, Wp, bp = (
        np.asarray(a, np.float32) for a in (x, Wq, bq, Wk, bk, Wv, bv, Wp, bp)
    )
    zb = not (np.any(bq) or np.any(bk) or np.any(bv))
    if _NC_CACHE is None or _NC_CACHE[1] != zb:
        _NC_CACHE = (build_bass(zero_bias=zb), zb)
    maps = _host_inputs(x, Wq, bq, Wk, bk, Wv, bv, Wp, bp, zb)
    res = run_bass_kernel_spmd(_NC_CACHE[0], maps, list(range(8)), trace=_trace)
    out = np.empty((B, T, C), np.float32)
    for b in range(B):
        acc = np.asarray(res.results[4 * b]["outT"], np.float32).copy()
        for g in range(1, 4):
            acc += np.asarray(res.results[4 * b + g]["outT"], np.float32)
        accT = np.concatenate(list(acc.reshape(NC4, C, 512)), axis=1)  # [C, T]
        out[b] = accT.T + bp[None, :]
    if _trace:
        return out, res
    return out
